# revision 1
# baseline (speedup 1.0000x reference)
"""Trainium2 Bass kernel for nn_Blur (upfirdn2d: up=2, pad=(2,1,2,1), 4-tap
separable filter [1,3,3,1] x [1,3,3,1] / 64).

Input  x [16, 128, 128, 128] f32  ->  Output [16, 128, 256, 256] f32.

Math (polyphase decomposition of the zero-insertion upsample + conv):
  per axis, even outputs:  y[2i]   = (1*x[i-1] + 3*x[i]) / 8
            odd  outputs:  y[2i+1] = (3*x[i]   + 1*x[i+1]) / 8
Separable 2D:
  pass 1 (vertical, on TensorE): V = A.T @ X with A the banded [128, 256]
     polyphase matrix carrying the full 1/64 scale, in float32r (single
     full-speed matmul; rel err ~1e-4, far inside the 2e-2 gate).
     (ROWPAIR_LOAD=True variant: 1KB input descriptors via K=64 split
     matmuls -- measured SLOWER on HW, 1KB packets stream at only
     ~13 GB/s vs 2x512B at ~18; left off.)
  pass 2 (horizontal): ScalarE computes u = 3V into SBUF (DVE may read
     at most one PSUM operand per instruction - NCC_IBVF027), then DVE
     adds  out[2j+k] = u[j] + V[j-1+2k]  read u(SBUF) + v(PSUM).

This kernel is DMA-bound: 84 MB/core over 16 DMA engines at ~22.5 GB/s
each ~= 242 us floor (235 us with 1KB input packets). Everything else is
sized to stay below that: balanced HWDGE queues (loads+stores alternate
between the SP and ACT rings), input loads issued LEAD groups ahead.

Sharding: pure data parallel, 2 examples per core x 8 cores. Each core
processes 256 channel-images of [128,128] in groups of 4 (matmul free dim
512).
"""

import numpy as np

H = 128
W = 128
N_CORES = 8
EX_PER_CORE = 2
NIMG_PER_CORE = EX_PER_CORE * 128  # 256 channel-images
GROUP = 4

# 2 input rows per SBUF partition (1KB DMA descriptors, K=64 x2 matmuls)
ROWPAIR_LOAD = False
# single fused DVE op per (eo, half) with k-interleaved contiguous writes
# (needs a zero-stride free dim on the u operand); False = 4 strided ops
FUSED_DVE = False


def _filter_matrix() -> np.ndarray:
    """A[h, m]: m in 0..127 -> even output row 2m; m in 128..255 -> odd row
    2(m-128)+1. Carries the full 1/64 scale of the separable pass."""
    A = np.zeros((H, 2 * H), np.float32)
    for i in range(H):
        # even output row 2i = (1*x[i-1] + 3*x[i])/64
        if i - 1 >= 0:
            A[i - 1, i] = 1.0 / 64
        A[i, i] = 3.0 / 64
        # odd output row 2i+1 = (3*x[i] + 1*x[i+1])/64
        A[i, H + i] = 3.0 / 64
        if i + 1 < H:
            A[i + 1, H + i] = 1.0 / 64
    return A


def filter_input() -> np.ndarray:
    A = _filter_matrix()
    if ROWPAIR_LOAD:
        # [64, 512]: partitions = row-pair index p; cols 0:256 = A[2p, :],
        # cols 256:512 = A[2p+1, :]  (both matmul operands keep base
        # partition 0 - matmul requires lhsT/rhs partition ranges to match)
        A = np.concatenate([A[0::2], A[1::2]], axis=1)
    return np.ascontiguousarray(A)


def build_kernel_body(tc, x, filt, out, nimg):
    """Emit the kernel IR. x [nimg,128,128] f32, filt [128,256] f32,
    out [nimg,256,256] f32."""
    from contextlib import ExitStack

    import concourse.mybir as mybir
    from concourse.ap import AP

    f32 = mybir.dt.float32
    f32r = mybir.dt.float32r
    nc = tc.nc
    ngroups = nimg // GROUP
    GW = GROUP * W  # 512

    LEAD = 6  # input loads issued this many groups ahead of use

    with ExitStack() as ctx:
        const_pool = ctx.enter_context(tc.tile_pool(name="const", bufs=1))
        xin_pool = ctx.enter_context(tc.tile_pool(name="xin", bufs=LEAD + 3))
        v_pool = ctx.enter_context(tc.tile_pool(name="v", bufs=4, space="PSUM"))
        o_pool = ctx.enter_context(tc.tile_pool(name="o", bufs=6))
        u_pool = ctx.enter_context(tc.tile_pool(name="u", bufs=4))

        A = const_pool.tile([64, 512] if ROWPAIR_LOAD else [128, 256], f32r)
        nc.scalar.dma_start(A[:], filt.bitcast(f32r))

        xg_tiles = {}

        def issue_load(gl):
            if gl >= ngroups:
                return
            j0 = gl * GROUP
            eng = nc.sync  # loads live alone on the SP ring -> deep prefetch
            if ROWPAIR_LOAD:
                xg = xin_pool.tile([64, 2 * GW], f32r)
                # partition p <- rows (2p, 2p+1): 1KB contiguous per (p, img)
                src = (
                    x[j0 : j0 + GROUP]
                    .rearrange("i (p r) w -> p i (r w)", r=2)
                    .bitcast(f32r)
                )
                dst = xg[:].rearrange("p (i rw) -> p i rw", i=GROUP)
            else:
                xg = xin_pool.tile([128, GW], f32r)
                src = x[j0 : j0 + GROUP].rearrange("i h w -> h i w").bitcast(f32r)
                dst = xg[:].rearrange("p (i w) -> p i w", i=GROUP)
            xg_tiles[gl] = xg
            eng.dma_start(dst, src)

        # startup burst: prefetch the first LEAD groups
        for gl in range(LEAD):
            issue_load(gl)

        for g in range(ngroups):
            issue_load(g + LEAD)
            i0 = g * GROUP
            xg = xg_tiles.pop(g)

            # pass 1 (vertical) on TensorE; partition p of v holds:
            #   cols 0:512   = V[2p,   (img, w)]   (even phase)
            #   cols 512:1024= V[2p+1, (img, w)]   (odd phase)
            v = v_pool.tile([128, 2 * GW], f32)
            if ROWPAIR_LOAD:
                xr = xg[:].rearrange("p (i r w) -> p r i w", r=2, i=GROUP)
                for eo in range(2):
                    m0 = eo * 128
                    nc.tensor.matmul(
                        v[:, eo * GW : (eo + 1) * GW],
                        A[:, m0 : m0 + 128],
                        xr[:, 0],
                        start=True,
                        stop=False,
                    )
                    nc.tensor.matmul(
                        v[:, eo * GW : (eo + 1) * GW],
                        A[:, 256 + m0 : 256 + m0 + 128],
                        xr[:, 1],
                        start=False,
                        stop=True,
                    )
            else:
                nc.tensor.matmul(v[:, 0:GW], A[:, 0:128], xg[:], start=True, stop=True)
                nc.tensor.matmul(
                    v[:, GW : 2 * GW], A[:, 128:256], xg[:], start=True, stop=True
                )

            # u = 3V on ScalarE (SBUF), so DVE reads one PSUM operand max
            u = u_pool.tile([128, 2 * GW], f32)
            nc.scalar.mul(u[:], v[:], 3.0)

            # out tile: partition p = output rows (2p, 2p+1):
            #   layout [img, eo, c] -> (c2 c) contiguous 2KB per (img)
            o = o_pool.tile([128, 2 * GROUP * 2 * W], f32)
            vE = v[:].rearrange("p (eo i w) -> p eo i w", eo=2, i=GROUP)
            vI = v[:].rearrange("p (eo i w) -> p i eo w", eo=2, i=GROUP)
            uE = u[:].rearrange("p (eo i w) -> p eo i w", eo=2, i=GROUP)
            uI = u[:].rearrange("p (eo i w) -> p i eo w", eo=2, i=GROUP)
            o4 = o[:].rearrange("p (i eo c) -> p i eo c", i=GROUP, eo=2)

            if FUSED_DVE:
                # one op per eo over (img, j=1..126, k=0..1):
                #   out[2j+k] = u[j] + V[j-1+2k]; contiguous writes
                ovv = o[:]
                uvv = u[:]
                vvv = v[:]
                pdim_o = list(ovv.ap[0])
                pdim_u = list(uvv.ap[0])
                pdim_v = list(vvv.ap[0])
                for eo in range(2):
                    out_ap = AP(
                        ovv.tensor,
                        ovv.offset + 256 * eo + 2,
                        [pdim_o, [512, GROUP], [2, 126], [1, 2]],
                    )
                    u_ap = AP(
                        uvv.tensor,
                        uvv.offset + 512 * eo + 1,
                        [pdim_u, [128, GROUP], [1, 126], [0, 2]],
                    )
                    v_ap = AP(
                        vvv.tensor,
                        vvv.offset + 512 * eo,
                        [pdim_v, [128, GROUP], [1, 126], [2, 2]],
                    )
                    nc.vector.tensor_add(out_ap, u_ap, v_ap)
            else:
                # interior: j = 1..126, col 2j+k = u[j] + V[j-1+2k]
                for eo in range(2):
                    for k in range(2):
                        nc.vector.tensor_add(
                            o4[:, :, eo, 2 + k : 254 + k : 2],
                            uE[:, eo, :, 1:127],
                            vE[:, eo, :, 2 * k : 2 * k + 126],
                        )
            # edge cols {1, 254}: 3*V[0]+V[1], 3*V[127]+V[126]
            nc.vector.tensor_add(
                o4[:, :, :, 1:255:253],
                uI[:, :, :, 0:128:127],
                vI[:, :, :, 1:127:125],
            )
            # seam cols {0, 255}: 3*V[0], 3*V[127] on ScalarE
            nc.scalar.copy(o4[:, :, :, 0:256:255], uI[:, :, :, 0:128:127])

            # one DMA for the whole group: partition p -> DRAM rows 2p, 2p+1
            # stores alternate between the two HWDGE rings (SP / ACT); on the
            # SP ring they sit AFTER the next input-load issue, so the LEAD
            # groups of slack keep prefetch from blocking behind them
            dst = out[i0 : i0 + GROUP].rearrange("i (p c2) c -> p i (c2 c)", c2=2)
            out_eng = nc.sync if g % 2 == 1 else nc.scalar
            out_eng.dma_start(dst, o[:].rearrange("p (i cc) -> p i cc", i=GROUP))


def build_bass(nimg=NIMG_PER_CORE, enable_asserts=False):
    import concourse.bacc as bacc
    import concourse.mybir as mybir
    import concourse.tile as tile

    f32 = mybir.dt.float32
    nc = bacc.Bacc(
        "TRN2",
        target_bir_lowering=False,
        debug=False,
        enable_asserts=enable_asserts,
        num_devices=N_CORES,
    )
    x = nc.dram_tensor("x", [nimg, H, W], f32, kind="ExternalInput").ap()
    fshape = [H // 2, 4 * H] if ROWPAIR_LOAD else [H, 2 * H]
    filt = nc.dram_tensor("filt", fshape, f32, kind="ExternalInput").ap()
    out = nc.dram_tensor("out", [nimg, 2 * H, 2 * W], f32, kind="ExternalOutput").ap()
    with tile.TileContext(nc) as tc:
        build_kernel_body(tc, x, filt, out, nimg)
    nc.compile()
    return nc


_NC_CACHE = {}


def kernel(x: np.ndarray, _trace=False, _trace_cores=None) -> np.ndarray:
    from concourse.bass_utils import run_bass_kernel_spmd

    assert x.shape == (16, 128, H, W), x.shape
    xf = np.ascontiguousarray(x, dtype=np.float32).reshape(N_CORES, NIMG_PER_CORE, H, W)
    A = filter_input()
    in_maps = [{"x": xf[k], "filt": A} for k in range(N_CORES)]

    key = NIMG_PER_CORE
    if key not in _NC_CACHE:
        _NC_CACHE[key] = build_bass()
    nc = _NC_CACHE[key]

    res = run_bass_kernel_spmd(
        nc,
        in_maps,
        core_ids=list(range(N_CORES)),
        trace=_trace,
        trace_cores=_trace_cores,
    )
    outs = np.stack([r["out"] for r in res.results])  # [8, 256, 256, 256]
    out = outs.reshape(16, 128, 2 * H, 2 * W)
    if _trace:
        kernel._last_result = res
    return out



# revision 3
# speedup vs baseline: 1.8675x; 1.8675x over previous
"""Trainium2 Bass kernel for nn_Blur (upfirdn2d: up=2, pad=(2,1,2,1), 4-tap
separable filter [1,3,3,1] x [1,3,3,1] / 64).

Input  x [16, 128, 128, 128] f32  ->  Output [16, 128, 256, 256] f32.

Math (polyphase decomposition of the zero-insertion upsample + conv), per
axis with zero boundary:
  even outputs:  y[2i]   = (1*x[i-1] + 3*x[i]) / 8
  odd  outputs:  y[2i+1] = (3*x[i]   + 1*x[i+1]) / 8

The kernel is HBM-bound, so all device I/O is bf16 (gate is rel_err < 2e-2;
the full bf16 rounding chain measures ~3e-3):
  in  8 MB/core + out 32 MB/core = 40 MB/core @ ~358 GB/s  ->  ~112 us floor
(f32 baseline moved 80 MB/core -> 269 us).

Pipeline per group of GROUP=8 channel-images:
  pass 1 (vertical) on TensorE: V = A.T @ X, A [128, 256] bf16 banded
     polyphase matrix carrying the full 1/64 scale; V in PSUM f32 laid out
     [p, (ph, i, j)] (ph = output row parity, p = row pair index).
  pass 2 (horizontal) split for engine balance (ACT ~ DVE ~ 109 us each,
     both under the DMA floor):
   - ACT: u = 3V -> SBUF bf16 (one op), plus part of Vb = copy(V) into
     zero-padded 130-wide blocks (pads persist across groups: K rotating
     tiles memset once at startup).
   - DVE: rest of the Vb copy (1x, PSUM operand), then the two adds in
     bf16 2x mode (all operands 16-bit SBUF stride-1, 4B-aligned):
        E[j] = u[j] + Vb[j-1]   (even output cols)
        O[j] = u[j] + Vb[j+1]   (odd  output cols)
     Zero pads make the j=0/127 boundaries fall out with no edge ops.
  store: one DMA per group, E|O planes contiguous 8 KB/partition bf16.

Host side: input is pre-permuted+cast to bf16 [32, h, i, w] per core so
loads are 2 KB/partition contiguous; output planes are gathered/interleaved
back to f32 NCHW with a threaded strided-cast pass.

Sharding: pure data parallel, 2 examples (256 channel-images) per core.
"""

import numpy as np

H = 128
W = 128
N_CORES = 8
NIMG_PER_CORE = 2 * 128  # 256 channel-images per core
GROUP = 8
NGROUPS = NIMG_PER_CORE // GROUP  # 32
NBLK = 2 * GROUP       # (ph, i) blocks per group
BLKW = W + 2           # padded Vb block width
LEAD = 4               # input loads issued this many groups ahead
VB_BUFS = 3            # persistent padded-Vb tiles (pads memset once)
ACT_VB_BLOCKS = 10     # Vb blocks copied by ACT; rest (NBLK-this) by DVE


def _filter_matrix() -> np.ndarray:
    """A[h, m] bf16-exact: m in 0..127 -> even output row 2m; m in 128..255
    -> odd row 2(m-128)+1. Carries the full 1/64 separable scale."""
    A = np.zeros((H, 2 * H), np.float32)
    for i in range(H):
        if i - 1 >= 0:
            A[i - 1, i] = 1.0 / 64
        A[i, i] = 3.0 / 64
        A[i, H + i] = 3.0 / 64
        if i + 1 < H:
            A[i + 1, H + i] = 1.0 / 64
    return A


def build_kernel_body(tc, x, filt, out, ngroups):
    """x [ngroups, 128, GROUP*W] bf16 (h-major, pre-permuted on host),
    filt [128, 256] bf16, out [ngroups, 128, 2*NBLK*W] bf16."""
    from contextlib import ExitStack

    import concourse.mybir as mybir

    bf16 = mybir.dt.bfloat16
    f32 = mybir.dt.float32
    nc = tc.nc
    GW = GROUP * W  # 1024

    with ExitStack() as ctx:
        const_pool = ctx.enter_context(tc.tile_pool(name="const", bufs=1))
        xin_pool = ctx.enter_context(tc.tile_pool(name="xin", bufs=LEAD + 2))
        v_pool = ctx.enter_context(tc.tile_pool(name="v", bufs=2, space="PSUM"))
        u_pool = ctx.enter_context(tc.tile_pool(name="u", bufs=3))
        vb_pool = ctx.enter_context(tc.tile_pool(name="vb", bufs=VB_BUFS))
        eo_pool = ctx.enter_context(tc.tile_pool(name="eo", bufs=3))

        A = const_pool.tile([H, 2 * H], bf16)
        nc.sync.dma_start(A[:], filt)

        # persistent padded-Vb tiles; zero the pad columns once
        vb_tiles = [
            vb_pool.tile([H, NBLK * BLKW], bf16, name=f"vb{i}")
            for i in range(VB_BUFS)
        ]
        for vb in vb_tiles:
            pads = vb[:].rearrange("p (b c) -> p b c", c=BLKW)[:, :, 0 : BLKW : BLKW - 1]
            nc.vector.memset(pads, 0.0)

        xg_tiles = {}

        def issue_load(gl):
            if gl >= ngroups:
                return
            xg = xin_pool.tile([H, GW], bf16)
            xg_tiles[gl] = xg
            nc.sync.dma_start(xg[:], x[gl])

        for gl in range(LEAD):
            issue_load(gl)

        for g in range(ngroups):
            issue_load(g + LEAD)
            xg = xg_tiles.pop(g)

            # pass 1 (vertical): V[p, (ph, i, j)] f32 in PSUM
            v = v_pool.tile([H, 2 * GW], f32)
            for ph in range(2):
                for half in range(2):
                    nc.tensor.matmul(
                        v[:, ph * GW + half * 512 : ph * GW + (half + 1) * 512],
                        A[:, ph * H : (ph + 1) * H],
                        xg[:, half * 512 : (half + 1) * 512],
                        start=True,
                        stop=True,
                    )
            v4 = v[:].rearrange("p (b j) -> p b j", b=NBLK)

            # u = 3V -> bf16 SBUF (ACT)
            u = u_pool.tile([H, 2 * GW], bf16)
            nc.scalar.mul(u[:], v[:], 3.0)

            # Vb = V -> bf16 into padded blocks, split ACT / DVE
            vb = vb_tiles[g % VB_BUFS]
            vb4 = vb[:].rearrange("p (b c) -> p b c", c=BLKW)
            ba = ACT_VB_BLOCKS
            if ba > 0:
                nc.scalar.copy(vb4[:, 0:ba, 1 : W + 1], v4[:, 0:ba, :])
            if ba < NBLK:
                nc.vector.tensor_copy(vb4[:, ba:NBLK, 1 : W + 1], v4[:, ba:NBLK, :])

            # pass 2 adds (DVE, bf16 2x): E = u + Vb[j-1], O = u + Vb[j+1]
            eo = eo_pool.tile([H, 2 * 2 * GW], bf16)
            eo4 = eo[:].rearrange("p (pl b j) -> p pl b j", pl=2, b=NBLK)
            u4 = u[:].rearrange("p (b j) -> p b j", b=NBLK)
            nc.vector.tensor_add(eo4[:, 0], u4[:, :, :], vb4[:, :, 0:W])
            nc.vector.tensor_add(eo4[:, 1], u4[:, :, :], vb4[:, :, 2 : W + 2])

            # one store per group: 8 KB/partition contiguous
            nc.sync.dma_start(out[g], eo[:])


def build_bass(ngroups=NGROUPS, enable_asserts=False):
    import concourse.bacc as bacc
    import concourse.mybir as mybir
    import concourse.tile as tile

    bf16 = mybir.dt.bfloat16
    nc = bacc.Bacc(
        "TRN2",
        target_bir_lowering=False,
        debug=False,
        enable_asserts=enable_asserts,
        num_devices=N_CORES,
    )
    x = nc.dram_tensor("x", [ngroups, H, GROUP * W], bf16, kind="ExternalInput").ap()
    filt = nc.dram_tensor("filt", [H, 2 * H], bf16, kind="ExternalInput").ap()
    out = nc.dram_tensor(
        "out", [ngroups, H, 2 * NBLK * W], bf16, kind="ExternalOutput"
    ).ap()
    with tile.TileContext(nc) as tc:
        build_kernel_body(tc, x, filt, out, ngroups)
    nc.compile()
    return nc


_NC_CACHE = {}


def _prep_input_core(x_core):
    """[256, 128, 128] f32 -> [32, h, i, w] bf16 contiguous."""
    import ml_dtypes

    v = x_core.reshape(NGROUPS, GROUP, H, W).transpose(0, 2, 1, 3)
    return np.ascontiguousarray(v.astype(ml_dtypes.bfloat16))


def _unpack_output_core(raw, dst):
    """raw [32, 128, 2*NBLK*W] bf16 -> dst [256, 256, 256] f32."""
    a = raw.reshape(NGROUPS, H, 2, 2, GROUP, W)  # g p pl ph i j
    # dst[g*GROUP+i, 2p+ph, 2j+pl]
    np.copyto(
        dst.reshape(NGROUPS, GROUP, H, 2, W, 2),
        a.transpose(0, 4, 1, 3, 5, 2),
        casting="unsafe",
    )


def kernel(x: np.ndarray, _trace=False, _trace_cores=None) -> np.ndarray:
    from concurrent.futures import ThreadPoolExecutor

    import ml_dtypes

    from concourse.bass_utils import run_bass_kernel_spmd

    assert x.shape == (16, 128, H, W), x.shape
    xf = np.ascontiguousarray(x, dtype=np.float32).reshape(
        N_CORES, NIMG_PER_CORE, H, W
    )
    A = _filter_matrix().astype(ml_dtypes.bfloat16)

    with ThreadPoolExecutor(N_CORES) as ex:
        xcores = list(ex.map(_prep_input_core, [xf[k] for k in range(N_CORES)]))
    in_maps = [{"x": xcores[k], "filt": A} for k in range(N_CORES)]

    key = NGROUPS
    if key not in _NC_CACHE:
        _NC_CACHE[key] = build_bass()
    nc = _NC_CACHE[key]

    res = run_bass_kernel_spmd(
        nc,
        in_maps,
        core_ids=list(range(N_CORES)),
        trace=_trace,
        trace_cores=_trace_cores,
    )
    out = np.empty((N_CORES * NIMG_PER_CORE, 2 * H, 2 * W), np.float32)
    with ThreadPoolExecutor(N_CORES) as ex:
        list(
            ex.map(
                lambda k: _unpack_output_core(
                    res.results[k]["out"],
                    out[k * NIMG_PER_CORE : (k + 1) * NIMG_PER_CORE],
                ),
                range(N_CORES),
            )
        )
    if _trace:
        kernel._last_result = res
    return out.reshape(16, 128, 2 * H, 2 * W)


# revision 5
# speedup vs baseline: 1.8825x; 1.0080x over previous
"""Trainium2 Bass kernel for nn_Blur (upfirdn2d: up=2, pad=(2,1,2,1), 4-tap
separable filter [1,3,3,1] x [1,3,3,1] / 64).

Input  x [16, 128, 128, 128] f32  ->  Output [16, 128, 256, 256] f32.

Math (polyphase decomposition of the zero-insertion upsample + conv), per
axis with zero boundary:
  even outputs:  y[2i]   = (1*x[i-1] + 3*x[i]) / 8
  odd  outputs:  y[2i+1] = (3*x[i]   + 1*x[i+1]) / 8

The kernel is HBM-bound, so all device I/O is bf16 (gate is rel_err < 2e-2;
the full bf16 rounding chain measures ~3e-3):
  in  8 MB/core + out 32 MB/core = 40 MB/core @ ~358 GB/s  ->  ~112 us floor
(f32 baseline moved 80 MB/core -> 269 us).

Pipeline per group of GROUP=8 channel-images:
  pass 1 (vertical) on TensorE: V = A.T @ X, A [128, 256] bf16 banded
     polyphase matrix carrying the full 1/64 scale; V in PSUM f32 laid out
     [p, (ph, i, j)] (ph = output row parity, p = row pair index).
  pass 2 (horizontal) split for engine balance (ACT ~ DVE ~ 109 us each,
     both under the DMA floor):
   - ACT: u = 3V -> SBUF bf16 (one op), plus part of Vb = copy(V) into
     zero-padded 130-wide blocks (pads persist across groups: K rotating
     tiles memset once at startup).
   - DVE: rest of the Vb copy (1x, PSUM operand), then the two adds in
     bf16 2x mode (all operands 16-bit SBUF stride-1, 4B-aligned):
        E[j] = u[j] + Vb[j-1]   (even output cols)
        O[j] = u[j] + Vb[j+1]   (odd  output cols)
     Zero pads make the j=0/127 boundaries fall out with no edge ops.
  store: one DMA per group, E|O planes contiguous 8 KB/partition bf16.

Host side: input is pre-permuted+cast to bf16 [32, h, i, w] per core so
loads are 2 KB/partition contiguous; output planes are gathered/interleaved
back to f32 NCHW with a threaded strided-cast pass.

Sharding: pure data parallel, 2 examples (256 channel-images) per core.
"""

import numpy as np

H = 128
W = 128
N_CORES = 8
NIMG_PER_CORE = 2 * 128  # 256 channel-images per core
GROUP = 8
NGROUPS = NIMG_PER_CORE // GROUP  # 32
NBLK = 2 * GROUP       # (ph, i) blocks per group
BLKW = W + 2           # padded Vb block width
LEAD = 6               # input loads issued this many groups ahead
VB_BUFS = 3            # persistent padded-Vb tiles (pads memset once)
ACT_VB_BLOCKS = 10     # Vb blocks copied by ACT; rest (NBLK-this) by DVE


def _filter_matrix() -> np.ndarray:
    """A[h, m] bf16-exact: m in 0..127 -> even output row 2m; m in 128..255
    -> odd row 2(m-128)+1. Carries the full 1/64 separable scale."""
    A = np.zeros((H, 2 * H), np.float32)
    for i in range(H):
        if i - 1 >= 0:
            A[i - 1, i] = 1.0 / 64
        A[i, i] = 3.0 / 64
        A[i, H + i] = 3.0 / 64
        if i + 1 < H:
            A[i + 1, H + i] = 1.0 / 64
    return A


def build_kernel_body(tc, x, filt, out, ngroups):
    """x [ngroups, 128, GROUP*W] bf16 (h-major, pre-permuted on host),
    filt [128, 256] bf16, out [ngroups, 128, 2*NBLK*W] bf16."""
    from contextlib import ExitStack

    import concourse.mybir as mybir

    bf16 = mybir.dt.bfloat16
    f32 = mybir.dt.float32
    nc = tc.nc
    GW = GROUP * W  # 1024

    with ExitStack() as ctx:
        const_pool = ctx.enter_context(tc.tile_pool(name="const", bufs=1))
        xin_pool = ctx.enter_context(tc.tile_pool(name="xin", bufs=LEAD + 2))
        v_pool = ctx.enter_context(tc.tile_pool(name="v", bufs=2, space="PSUM"))
        u_pool = ctx.enter_context(tc.tile_pool(name="u", bufs=4))
        vb_pool = ctx.enter_context(tc.tile_pool(name="vb", bufs=VB_BUFS))
        eo_pool = ctx.enter_context(tc.tile_pool(name="eo", bufs=4))

        A = const_pool.tile([H, 2 * H], bf16)
        nc.sync.dma_start(A[:], filt)

        # persistent padded-Vb tiles; zero the pad columns once
        vb_tiles = [
            vb_pool.tile([H, NBLK * BLKW], bf16, name=f"vb{i}")
            for i in range(VB_BUFS)
        ]
        for vb in vb_tiles:
            pads = vb[:].rearrange("p (b c) -> p b c", c=BLKW)[:, :, 0 : BLKW : BLKW - 1]
            nc.vector.memset(pads, 0.0)

        xg_tiles = {}

        def issue_load(gl):
            if gl >= ngroups:
                return
            xg = xin_pool.tile([H, GW], bf16)
            xg_tiles[gl] = xg
            # loads ride the ACT HWDGE ring (trigger never waits: the xin
            # buffer was freed LEAD+2 groups ago), so prefetch cannot get
            # stuck behind a store blocked on compute on the SP ring
            nc.scalar.dma_start(xg[:], x[gl])

        for gl in range(LEAD):
            issue_load(gl)

        for g in range(ngroups):
            issue_load(g + LEAD)
            xg = xg_tiles.pop(g)

            # pass 1 (vertical): V[p, (ph, i, j)] f32 in PSUM
            v = v_pool.tile([H, 2 * GW], f32)
            for ph in range(2):
                for half in range(2):
                    nc.tensor.matmul(
                        v[:, ph * GW + half * 512 : ph * GW + (half + 1) * 512],
                        A[:, ph * H : (ph + 1) * H],
                        xg[:, half * 512 : (half + 1) * 512],
                        start=True,
                        stop=True,
                    )
            v4 = v[:].rearrange("p (b j) -> p b j", b=NBLK)

            # u = 3V -> bf16 SBUF (ACT)
            u = u_pool.tile([H, 2 * GW], bf16)
            nc.scalar.mul(u[:], v[:], 3.0)

            # Vb = V -> bf16 into padded blocks, split ACT / DVE
            vb = vb_tiles[g % VB_BUFS]
            vb4 = vb[:].rearrange("p (b c) -> p b c", c=BLKW)
            ba = ACT_VB_BLOCKS
            if ba > 0:
                nc.scalar.copy(vb4[:, 0:ba, 1 : W + 1], v4[:, 0:ba, :])
            if ba < NBLK:
                nc.vector.tensor_copy(vb4[:, ba:NBLK, 1 : W + 1], v4[:, ba:NBLK, :])

            # pass 2 adds (DVE, bf16 2x): E = u + Vb[j-1], O = u + Vb[j+1]
            eo = eo_pool.tile([H, 2 * 2 * GW], bf16)
            eo4 = eo[:].rearrange("p (pl b j) -> p pl b j", pl=2, b=NBLK)
            u4 = u[:].rearrange("p (b j) -> p b j", b=NBLK)
            nc.vector.tensor_add(eo4[:, 0], u4[:, :, :], vb4[:, :, 0:W])
            nc.vector.tensor_add(eo4[:, 1], u4[:, :, :], vb4[:, :, 2 : W + 2])

            # one store per group: 8 KB/partition contiguous
            nc.sync.dma_start(out[g], eo[:])


def build_bass(ngroups=NGROUPS, enable_asserts=False):
    import concourse.bacc as bacc
    import concourse.mybir as mybir
    import concourse.tile as tile

    bf16 = mybir.dt.bfloat16
    nc = bacc.Bacc(
        "TRN2",
        target_bir_lowering=False,
        debug=False,
        enable_asserts=enable_asserts,
        num_devices=N_CORES,
    )
    x = nc.dram_tensor("x", [ngroups, H, GROUP * W], bf16, kind="ExternalInput").ap()
    filt = nc.dram_tensor("filt", [H, 2 * H], bf16, kind="ExternalInput").ap()
    out = nc.dram_tensor(
        "out", [ngroups, H, 2 * NBLK * W], bf16, kind="ExternalOutput"
    ).ap()
    with tile.TileContext(nc) as tc:
        build_kernel_body(tc, x, filt, out, ngroups)
    nc.compile()
    return nc


_NC_CACHE = {}


def _prep_input_core(x_core):
    """[256, 128, 128] f32 -> [32, h, i, w] bf16 contiguous."""
    import ml_dtypes

    v = x_core.reshape(NGROUPS, GROUP, H, W).transpose(0, 2, 1, 3)
    return np.ascontiguousarray(v.astype(ml_dtypes.bfloat16))


def _unpack_output_core(raw, dst):
    """raw [32, 128, 2*NBLK*W] bf16 -> dst [256, 256, 256] f32."""
    a = raw.reshape(NGROUPS, H, 2, 2, GROUP, W)  # g p pl ph i j
    # dst[g*GROUP+i, 2p+ph, 2j+pl]
    np.copyto(
        dst.reshape(NGROUPS, GROUP, H, 2, W, 2),
        a.transpose(0, 4, 1, 3, 5, 2),
        casting="unsafe",
    )


def kernel(x: np.ndarray, _trace=False, _trace_cores=None) -> np.ndarray:
    from concurrent.futures import ThreadPoolExecutor

    import ml_dtypes

    from concourse.bass_utils import run_bass_kernel_spmd

    assert x.shape == (16, 128, H, W), x.shape
    xf = np.ascontiguousarray(x, dtype=np.float32).reshape(
        N_CORES, NIMG_PER_CORE, H, W
    )
    A = _filter_matrix().astype(ml_dtypes.bfloat16)

    with ThreadPoolExecutor(N_CORES) as ex:
        xcores = list(ex.map(_prep_input_core, [xf[k] for k in range(N_CORES)]))
    in_maps = [{"x": xcores[k], "filt": A} for k in range(N_CORES)]

    key = NGROUPS
    if key not in _NC_CACHE:
        _NC_CACHE[key] = build_bass()
    nc = _NC_CACHE[key]

    res = run_bass_kernel_spmd(
        nc,
        in_maps,
        core_ids=list(range(N_CORES)),
        trace=_trace,
        trace_cores=_trace_cores,
    )
    out = np.empty((N_CORES * NIMG_PER_CORE, 2 * H, 2 * W), np.float32)
    with ThreadPoolExecutor(N_CORES) as ex:
        list(
            ex.map(
                lambda k: _unpack_output_core(
                    res.results[k]["out"],
                    out[k * NIMG_PER_CORE : (k + 1) * NIMG_PER_CORE],
                ),
                range(N_CORES),
            )
        )
    if _trace:
        kernel._last_result = res
    return out.reshape(16, 128, 2 * H, 2 * W)


# revision 6
# speedup vs baseline: 1.9241x; 1.0221x over previous
"""Trainium2 Bass kernel for nn_Blur (upfirdn2d: up=2, pad=(2,1,2,1), 4-tap
separable filter [1,3,3,1] x [1,3,3,1] / 64).

Input  x [16, 128, 128, 128] f32  ->  Output [16, 128, 256, 256] f32.

Math (polyphase decomposition of the zero-insertion upsample + conv), per
axis with zero boundary:
  even outputs:  y[2i]   = (1*x[i-1] + 3*x[i]) / 8
  odd  outputs:  y[2i+1] = (3*x[i]   + 1*x[i+1]) / 8

The kernel is HBM-bound, so all device I/O is bf16 (gate is rel_err < 2e-2;
the full bf16 rounding chain measures ~3e-3):
  in  8 MB/core + out 32 MB/core = 40 MB/core @ ~358 GB/s  ->  ~112 us floor
(f32 baseline moved 80 MB/core -> 269 us).

Pipeline per group of GROUP=8 channel-images:
  pass 1 (vertical) on TensorE: V = A.T @ X, A [128, 256] bf16 banded
     polyphase matrix carrying the full 1/64 scale; V in PSUM f32 laid out
     [p, (ph, i, j)] (ph = output row parity, p = row pair index).
  pass 2 (horizontal) split for engine balance (ACT ~ DVE ~ 109 us each,
     both under the DMA floor):
   - ACT: u = 3V -> SBUF bf16 (one op), plus part of Vb = copy(V) into
     zero-padded 130-wide blocks (pads persist across groups: K rotating
     tiles memset once at startup).
   - DVE: rest of the Vb copy (1x, PSUM operand), then the two adds in
     bf16 2x mode (all operands 16-bit SBUF stride-1, 4B-aligned):
        E[j] = u[j] + Vb[j-1]   (even output cols)
        O[j] = u[j] + Vb[j+1]   (odd  output cols)
     Zero pads make the j=0/127 boundaries fall out with no edge ops.
  store: one DMA per group, E|O planes contiguous 8 KB/partition bf16.

Host side: input is pre-permuted+cast to bf16 [32, h, i, w] per core so
loads are 2 KB/partition contiguous; output planes are gathered/interleaved
back to f32 NCHW with a threaded strided-cast pass.

Sharding: pure data parallel, 2 examples (256 channel-images) per core.
"""

import numpy as np

H = 128
W = 128
N_CORES = 8
NIMG_PER_CORE = 2 * 128  # 256 channel-images per core
GROUP = 8
NGROUPS = NIMG_PER_CORE // GROUP  # 32
NBLK = 2 * GROUP       # (ph, i) blocks per group
BLKW = W + 2           # padded Vb block width
LEAD = 12              # input loads issued this many groups ahead
VB_BUFS = 3            # persistent padded-Vb tiles (pads memset once)
ACT_VB_BLOCKS = 10     # Vb blocks copied by ACT; rest (NBLK-this) by DVE


def _filter_matrix() -> np.ndarray:
    """A[h, m] bf16-exact: m in 0..127 -> even output row 2m; m in 128..255
    -> odd row 2(m-128)+1. Carries the full 1/64 separable scale."""
    A = np.zeros((H, 2 * H), np.float32)
    for i in range(H):
        if i - 1 >= 0:
            A[i - 1, i] = 1.0 / 64
        A[i, i] = 3.0 / 64
        A[i, H + i] = 3.0 / 64
        if i + 1 < H:
            A[i + 1, H + i] = 1.0 / 64
    return A


def build_kernel_body(tc, x, filt, out, ngroups):
    """x [ngroups, 128, GROUP*W] bf16 (h-major, pre-permuted on host),
    filt [128, 256] bf16, out [ngroups, 128, 2*NBLK*W] bf16."""
    from contextlib import ExitStack

    import concourse.mybir as mybir

    bf16 = mybir.dt.bfloat16
    f32 = mybir.dt.float32
    nc = tc.nc
    GW = GROUP * W  # 1024

    with ExitStack() as ctx:
        const_pool = ctx.enter_context(tc.tile_pool(name="const", bufs=1))
        xin_pool = ctx.enter_context(tc.tile_pool(name="xin", bufs=LEAD + 2))
        v_pool = ctx.enter_context(tc.tile_pool(name="v", bufs=2, space="PSUM"))
        u_pool = ctx.enter_context(tc.tile_pool(name="u", bufs=4))
        vb_pool = ctx.enter_context(tc.tile_pool(name="vb", bufs=VB_BUFS))
        eo_pool = ctx.enter_context(tc.tile_pool(name="eo", bufs=4))

        A = const_pool.tile([H, 2 * H], bf16)
        nc.sync.dma_start(A[:], filt)

        # persistent padded-Vb tiles; zero the pad columns once
        vb_tiles = [
            vb_pool.tile([H, NBLK * BLKW], bf16, name=f"vb{i}")
            for i in range(VB_BUFS)
        ]
        for vb in vb_tiles:
            pads = vb[:].rearrange("p (b c) -> p b c", c=BLKW)[:, :, 0 : BLKW : BLKW - 1]
            nc.vector.memset(pads, 0.0)

        xg_tiles = {}

        def issue_load(gl):
            if gl >= ngroups:
                return
            xg = xin_pool.tile([H, GW], bf16)
            xg_tiles[gl] = xg
            # loads ride the ACT HWDGE ring (trigger never waits: the xin
            # buffer was freed LEAD+2 groups ago), so prefetch cannot get
            # stuck behind a store blocked on compute on the SP ring
            nc.scalar.dma_start(xg[:], x[gl])

        for gl in range(LEAD):
            issue_load(gl)

        for g in range(ngroups):
            issue_load(g + LEAD)
            xg = xg_tiles.pop(g)

            # pass 1 (vertical): V[p, (ph, i, j)] f32 in PSUM
            v = v_pool.tile([H, 2 * GW], f32)
            for ph in range(2):
                for half in range(2):
                    nc.tensor.matmul(
                        v[:, ph * GW + half * 512 : ph * GW + (half + 1) * 512],
                        A[:, ph * H : (ph + 1) * H],
                        xg[:, half * 512 : (half + 1) * 512],
                        start=True,
                        stop=True,
                    )
            v4 = v[:].rearrange("p (b j) -> p b j", b=NBLK)

            # u = 3V -> bf16 SBUF (ACT)
            u = u_pool.tile([H, 2 * GW], bf16)
            nc.scalar.mul(u[:], v[:], 3.0)

            # Vb = V -> bf16 into padded blocks, split ACT / DVE
            vb = vb_tiles[g % VB_BUFS]
            vb4 = vb[:].rearrange("p (b c) -> p b c", c=BLKW)
            ba = ACT_VB_BLOCKS
            if ba > 0:
                nc.scalar.copy(vb4[:, 0:ba, 1 : W + 1], v4[:, 0:ba, :])
            if ba < NBLK:
                nc.vector.tensor_copy(vb4[:, ba:NBLK, 1 : W + 1], v4[:, ba:NBLK, :])

            # pass 2 adds (DVE, bf16 2x): E = u + Vb[j-1], O = u + Vb[j+1]
            eo = eo_pool.tile([H, 2 * 2 * GW], bf16)
            eo4 = eo[:].rearrange("p (pl b j) -> p pl b j", pl=2, b=NBLK)
            u4 = u[:].rearrange("p (b j) -> p b j", b=NBLK)
            nc.vector.tensor_add(eo4[:, 0], u4[:, :, :], vb4[:, :, 0:W])
            nc.vector.tensor_add(eo4[:, 1], u4[:, :, :], vb4[:, :, 2 : W + 2])

            # one store per group: 8 KB/partition contiguous
            nc.sync.dma_start(out[g], eo[:])


def build_bass(ngroups=NGROUPS, enable_asserts=False):
    import concourse.bacc as bacc
    import concourse.mybir as mybir
    import concourse.tile as tile

    bf16 = mybir.dt.bfloat16
    nc = bacc.Bacc(
        "TRN2",
        target_bir_lowering=False,
        debug=False,
        enable_asserts=enable_asserts,
        num_devices=N_CORES,
    )
    x = nc.dram_tensor("x", [ngroups, H, GROUP * W], bf16, kind="ExternalInput").ap()
    filt = nc.dram_tensor("filt", [H, 2 * H], bf16, kind="ExternalInput").ap()
    out = nc.dram_tensor(
        "out", [ngroups, H, 2 * NBLK * W], bf16, kind="ExternalOutput"
    ).ap()
    with tile.TileContext(nc) as tc:
        build_kernel_body(tc, x, filt, out, ngroups)
    nc.compile()
    return nc


_NC_CACHE = {}


def _prep_input_core(x_core):
    """[256, 128, 128] f32 -> [32, h, i, w] bf16 contiguous."""
    import ml_dtypes

    v = x_core.reshape(NGROUPS, GROUP, H, W).transpose(0, 2, 1, 3)
    return np.ascontiguousarray(v.astype(ml_dtypes.bfloat16))


def _unpack_output_core(raw, dst):
    """raw [32, 128, 2*NBLK*W] bf16 -> dst [256, 256, 256] f32."""
    a = raw.reshape(NGROUPS, H, 2, 2, GROUP, W)  # g p pl ph i j
    # dst[g*GROUP+i, 2p+ph, 2j+pl]
    np.copyto(
        dst.reshape(NGROUPS, GROUP, H, 2, W, 2),
        a.transpose(0, 4, 1, 3, 5, 2),
        casting="unsafe",
    )


def kernel(x: np.ndarray, _trace=False, _trace_cores=None) -> np.ndarray:
    from concurrent.futures import ThreadPoolExecutor

    import ml_dtypes

    from concourse.bass_utils import run_bass_kernel_spmd

    assert x.shape == (16, 128, H, W), x.shape
    xf = np.ascontiguousarray(x, dtype=np.float32).reshape(
        N_CORES, NIMG_PER_CORE, H, W
    )
    A = _filter_matrix().astype(ml_dtypes.bfloat16)

    with ThreadPoolExecutor(N_CORES) as ex:
        xcores = list(ex.map(_prep_input_core, [xf[k] for k in range(N_CORES)]))
    in_maps = [{"x": xcores[k], "filt": A} for k in range(N_CORES)]

    key = NGROUPS
    if key not in _NC_CACHE:
        _NC_CACHE[key] = build_bass()
    nc = _NC_CACHE[key]

    res = run_bass_kernel_spmd(
        nc,
        in_maps,
        core_ids=list(range(N_CORES)),
        trace=_trace,
        trace_cores=_trace_cores,
    )
    out = np.empty((N_CORES * NIMG_PER_CORE, 2 * H, 2 * W), np.float32)
    with ThreadPoolExecutor(N_CORES) as ex:
        list(
            ex.map(
                lambda k: _unpack_output_core(
                    res.results[k]["out"],
                    out[k * NIMG_PER_CORE : (k + 1) * NIMG_PER_CORE],
                ),
                range(N_CORES),
            )
        )
    if _trace:
        kernel._last_result = res
    return out.reshape(16, 128, 2 * H, 2 * W)
